# revision 2
# baseline (speedup 1.0000x reference)
"""Trainium2 Bass kernel for nn_Encoder_44736379355603 (2-layer GCN encoder).

v2: SBUF-resident gather tables. The per-edge random access (3.3M edges x
2 layers) runs as SBUF-source dma_gather (transpose mode, bf16 pair tokens)
instead of per-edge 256B random HBM reads -- the HBM small-descriptor
penalty is the baseline's bottleneck.

Math (PyG GCNConv with self-loops, eval mode):
    deg = in-degree over (edges + self-loops); dis = deg^-1/2
    g1 = dis * (x @ W1)                       # [N, 64] bf16 table in SBUF
    h  = relu(dis * A_sum(g1) + b1)           # A_sum = segment_sum over in-edges
    g2 = dis * (h @ [W_mu | W_ls])            # [N, 32] bf16, AllGather
    mu|ls = dis * A_sum(g2) + [b_mu | b_ls]

Device mapping (8 NeuronCores, single SPMD program):
  - dst nodes sharded: core c owns rows [c*12544, (c+1)*12544).
  - Tables: bf16 pair tokens (256B = 2 nodes); 2 src-halves per layer so
    token ids fit int16. Layer1 token = [g1 even | g1 odd]; layer2 token =
    [g2 even | g2 odd | 64B pad]. Table half = [128 part, 196 ranks x 256B];
    token q at partition q%128, rank q//128.
  - Gather: SBUF-source dma_gather transpose mode -> channel-major
    [128 words, n edges]; PE-transpose back to edge-major, ACT copies
    PSUM->SBUF (batched x4 cols), DVE copy_predicated resolves parity
    in-place, one one-hot matmul per 128-edge column accumulates the
    segment-sum in PSUM. Both layers share identical idx/did/par streams.
  - Between layers: AllGather of bf16 g2 shard via DRAM, then one strided
    DMA per half loads the L2 table (reusing the L1 table SBUF storage).
"""
import numpy as np

P = 128
CH1 = 64          # layer-1 channels
CH2 = 32          # layer-2 channels (mu|ls premixed)
GCHUNK = 4096     # idxs per dma_gather instruction
OHB = 8           # one-hot builds batched per DVE op
TB = 4            # transposed columns per ACT copy / predication batch
N_CORES = 8


class Plan:
    """Host-side graph preprocessing producing uniform per-core schedules."""

    def __init__(self, n_nodes, n_cores, edge_src, edge_dst):
        self.n_cores = n_cores
        shard = -(-n_nodes // (n_cores * P)) * P
        npad = shard * n_cores
        assert npad % 512 == 0
        half = npad // 2
        pairs_half = half // 2
        assert pairs_half % P == 0 and pairs_half - 1 < 32768
        self.shard, self.npad, self.half, self.pairs_half = (
            shard, npad, half, pairs_half)
        self.nwin = shard // P
        self.ranks_half = pairs_half // P          # table ranks per half

        deg = np.bincount(edge_dst, minlength=n_nodes) + 1
        self.dis = np.zeros(npad, np.float32)
        self.dis[:n_nodes] = (1.0 / np.sqrt(deg.astype(np.float64))).astype(
            np.float32)

        loops = np.arange(n_nodes, dtype=np.int64)
        src = np.concatenate([edge_src, loops])
        dst = np.concatenate([edge_dst, loops])

        core = dst // shard
        w = (dst % shard) // P
        hh = src // half
        order = np.lexsort((dst, hh, core))
        src, dst, core, hh, w = src[order], dst[order], core[order], hh[order], w[order]

        cnt = np.zeros((n_cores, 2, self.nwin), np.int64)
        np.add.at(cnt, (core, hh, w), 1)
        self.pwh = (-(-cnt.max(axis=0) // P) * P).astype(np.int64)   # [2, nwin]
        self.stream_off = np.zeros((2, self.nwin), np.int64)
        for h in range(2):
            self.stream_off[h] = np.cumsum(self.pwh[h]) - self.pwh[h]
        self.stream_len = self.pwh.sum(axis=1)
        self.chunks = []
        for h in range(2):
            L, ch = int(self.stream_len[h]), []
            while L > 0:
                s = min(GCHUNK, L)
                ch.append(s)
                L -= s
            self.chunks.append(ch)

        self.per_core = []
        for c in range(n_cores):
            m = core == c
            sc, dc, qc, wc = src[m], dst[m], hh[m], w[m]
            run_key = qc * self.nwin + wc
            starts = np.flatnonzero(np.diff(run_key, prepend=-1))
            rank = np.arange(len(run_key)) - np.repeat(
                np.arange(len(run_key))[starts],
                np.diff(starts, append=len(run_key)))
            pos = self.stream_off[qc, wc] + rank
            self.per_core.append((sc, dc, qc, wc, pos))

        tot_cols = int(self.pwh.sum()) // P
        self.ne = -(-tot_cols // 128) * 128

    def core_arrays(self, c):
        """idx streams (int16, one per half) + did/par arrays for core c."""
        sc, dc, qc, wc, pos = self.per_core[c]
        shard, pairs_half = self.shard, self.pairs_half
        idx, dids, pars = [], [], []
        for h in range(2):
            L = int(self.stream_len[h])
            ii = np.zeros(L, np.int16)
            dd = np.full(L, -1.0, np.float32)
            pp = np.zeros(L, np.float32)
            m = qc == h
            p_, s_, d_ = pos[m], sc[m], dc[m]
            ii[p_] = ((s_ >> 1) % pairs_half).astype(np.int16)
            dd[p_] = (d_ - c * shard - wc[m] * P).astype(np.float32)
            pp[p_] = (s_ & 1).astype(np.float32)
            idx.append(ii)
            dids.append(dd)
            pars.append(pp)

        da, pa = [], []
        for w in range(self.nwin):
            for h in range(2):
                o = int(self.stream_off[h, w])
                n = int(self.pwh[h, w])
                da.append(dids[h][o:o + n].reshape(-1, P))
                pa.append(pars[h][o:o + n].reshape(-1, P))
        da = np.concatenate(da)
        pa = np.concatenate(pa)
        da = np.concatenate([da, np.full((self.ne - len(da), P), -1.0,
                                         np.float32)])
        pa = np.concatenate([pa, np.zeros((self.ne - len(pa), P),
                                          np.float32)])
        return idx, da.T.copy(), pa.T.copy()


def _wrap16(a):
    n = a.shape[0]
    assert n % 16 == 0
    w = a.reshape(n // 16, 16).T.astype(np.int16)
    return np.tile(w, (8, 1))


def build_program(plan, n_cores, rep=1, n_ag=1,
                  stages=('b1', 'l1', 'ag', 'b2', 'l2')):
    """rep>1 builds a timing variant: n_ag AllGathers up front, then the
    pipeline (minus AllGather) repeated `rep` times inside a For_i loop."""
    import concourse.bass as bass
    import concourse.bacc as bacc
    import concourse.mybir as mybir
    import concourse.tile as tile

    shard, npad, nwin = plan.shard, plan.npad, plan.nwin
    half, pairs_half, R = plan.half, plan.pairs_half, plan.ranks_half
    f32 = mybir.dt.float32
    bf16 = mybir.dt.bfloat16
    i16 = mybir.dt.int16
    Copy = mybir.ActivationFunctionType.Copy

    nc = bacc.Bacc("TRN2", target_bir_lowering=False, debug=False,
                   enable_asserts=False, num_devices=n_cores,
                   num_swdge_queues=1)

    XB = 3584 if npad % 3584 == 0 else 512
    assert XB % 256 == 0
    nxb = npad // XB
    rpt = XB // 256                      # table ranks per x tile
    totR = npad // 256                   # total table ranks (both halves)
    assert R % rpt == 0

    xT = nc.dram_tensor("xT", [nxb * P, XB], bf16, kind="ExternalInput")
    disP = nc.dram_tensor("disP", [P, 2 * totR], f32, kind="ExternalInput")
    W1 = nc.dram_tensor("W1", [P, CH1], bf16, kind="ExternalInput")
    Wc = nc.dram_tensor("Wc", [CH1, CH2], bf16, kind="ExternalInput")
    b1r = nc.dram_tensor("b1r", [P, CH1], f32, kind="ExternalInput")
    bcr = nc.dram_tensor("bcr", [P, CH2], f32, kind="ExternalInput")
    iot = nc.dram_tensor("iot", [P, P], bf16, kind="ExternalInput")
    idn = nc.dram_tensor("idn", [P, P], bf16, kind="ExternalInput")
    disW = nc.dram_tensor("disW", [P, nwin], f32, kind="ExternalInput")
    idxs = [nc.dram_tensor(f"idx_{h}", [len(plan.chunks[h]) * P, GCHUNK // 16],
                           i16, kind="ExternalInput") for h in range(2)]
    didT = nc.dram_tensor("didT", [(plan.ne // 128) * P, 128], bf16,
                          kind="ExternalInput")
    parT = nc.dram_tensor("parT", [(plan.ne // 128) * P, 128], mybir.dt.uint8,
                          kind="ExternalInput")
    out_c = nc.dram_tensor("out_c", [shard, CH2], f32, kind="ExternalOutput")

    g2c = nc.dram_tensor("g2c", [shard, CH2], bf16, kind="Internal")
    g2f = nc.dram_tensor("g2f", [npad, CH2], bf16, kind="Internal")

    with tile.TileContext(nc) as tc:
        with (
            tc.tile_pool(name="const", bufs=1) as cpool,
            tc.tile_pool(name="tab", bufs=1) as tpool,
            tc.tile_pool(name="xload", bufs=1) as xpool,
            tc.tile_pool(name="gat", bufs=1) as gpool,
            tc.tile_pool(name="oh", bufs=1) as ohpool,
            tc.tile_pool(name="didp", bufs=1) as dpool,
            tc.tile_pool(name="ixp", bufs=1) as ipool,
            tc.tile_pool(name="trp", bufs=1) as rpool,
            tc.tile_pool(name="epi", bufs=1) as epool,
            tc.tile_pool(name="ps", bufs=1, space="PSUM") as pspool,
        ):
            W1_t = cpool.tile([P, CH1], bf16, name="W1_t")
            Wc_t = cpool.tile([CH1, CH2], bf16, name="Wc_t")
            b1_t = cpool.tile([P, CH1], f32, name="b1_t")
            bc_t = cpool.tile([P, CH2], f32, name="bc_t")
            io_t = cpool.tile([P, P], bf16, name="io_t")
            id_t = cpool.tile([P, P], bf16, name="id_t")
            dW_t = cpool.tile([P, nwin], f32, name="dW_t")
            dP_t = cpool.tile([P, 2 * totR], f32, name="dP_t")
            for t, d in ((W1_t, W1), (Wc_t, Wc), (b1_t, b1r), (bc_t, bcr),
                         (io_t, iot), (id_t, idn), (dW_t, disW), (dP_t, disP)):
                nc.sync.dma_start(out=t[:], in_=d[:, :])

            def table_tiles():
                return [tpool.tile([P, R * 128], bf16, tag=f"tab{H}", bufs=1,
                                   name=f"tab{H}") for H in range(2)]

            def emit_b1():
                """x @ W1 -> dis-scaled bf16 pair-token tables (both halves)."""
                tabs = table_tiles()
                for b in range(nxb):
                    xt = xpool.tile([P, XB], bf16, tag="xt", bufs=2, name="xt")
                    nc.sync.dma_start(out=xt[:], in_=xT[b * P:(b + 1) * P, :])
                    for g in range(rpt):
                        gg = b * rpt + g
                        H, gl = gg // R, gg % R
                        ps = pspool.tile([P, P], f32, tag="bld", bufs=2,
                                         name="bld")
                        nc.tensor.matmul(out=ps[:, 0:CH1],
                                         lhsT=xt[:, g * 256:g * 256 + 128],
                                         rhs=W1_t[:], start=True, stop=True)
                        nc.tensor.matmul(out=ps[:, CH1:128],
                                         lhsT=xt[:, g * 256 + 128:(g + 1) * 256],
                                         rhs=W1_t[:], start=True, stop=True)
                        nc.vector.tensor_tensor(
                            out=tabs[H][:, gl * 128:(gl + 1) * 128].rearrange(
                                "p (t c) -> p t c", t=2),
                            in0=ps[:].rearrange("p (t c) -> p t c", t=2),
                            in1=dP_t[:, 2 * gg:2 * gg + 2].rearrange(
                                "p (t c) -> p t c", c=1).to_broadcast(
                                    [P, 2, CH1]),
                            op=mybir.AluOpType.mult)
                return tabs

            def emit_b2():
                """Load AllGathered bf16 g2 into L2 tables (reuses storage)."""
                tabs = table_tiles()
                for H in range(2):
                    nc.vector.memset(
                        tabs[H][:].rearrange("p (g e) -> p g e",
                                             e=128)[:, :, 2 * CH2:128], 0)
                    nc.sync.dma_start(
                        out=tabs[H][:].rearrange("p (g e) -> p g e",
                                                 e=128)[:, :, 0:2 * CH2],
                        in_=g2f[H * half:(H + 1) * half, :].rearrange(
                            "(g p t) c -> p g (t c)", p=P, t=2))
                return tabs

            def propagate(layer, tabs):
                ch = CH1 if layer == 1 else CH2
                gst = [{"q": [], "next": 0, "start": 0} for _ in range(2)]
                dtiles = {}
                dstate = {"oh": None, "oh_e": -1}

                def emit_chunk(h):
                    st = gst[h]
                    if st["next"] >= len(plan.chunks[h]):
                        return
                    ck = st["next"]
                    n = plan.chunks[h][ck]
                    s0 = st["start"]
                    ix = ipool.tile([P, GCHUNK // 16], i16,
                                    tag=f"ix{h}", bufs=2, name=f"ix{h}")
                    nc.sync.dma_start(out=ix[:],
                                      in_=idxs[h][ck * P:(ck + 1) * P, :])
                    gt = gpool.tile([P, GCHUNK], bf16,
                                    tag=f"g{h}", bufs=2, name=f"g{h}")
                    nc.gpsimd.dma_gather(
                        out_ap=gt[:, :n].rearrange("p (a n) -> p a n", a=1),
                        in_ap=tabs[h][:],
                        idxs_ap=ix[:, :n // 16],
                        num_idxs=n, num_idxs_reg=n, elem_size=128,
                        transpose=True, single_packet=False, queue_num=0,
                        sbuf_tokens_per_rank=128,
                        sbuf_free_dim_per_rank=256,
                        sbuf_free_dim_pad_per_rank=0,
                        sbuf_byte_offset=0)
                    st["q"].append((s0, s0 + n, gt))
                    st["next"] += 1
                    st["start"] += n

                def covers(h, pos):
                    st = gst[h]
                    return st["q"] and st["q"][0][0] <= pos < st["q"][0][1]

                def lookup(h, pos):
                    st = gst[h]
                    while st["q"] and st["q"][0][1] <= pos:
                        st["q"].pop(0)
                    while not covers(h, pos):
                        emit_chunk(h)
                        while st["q"] and st["q"][0][1] <= pos:
                            st["q"].pop(0)
                    return st["q"][0]

                def load_dtile(tt):
                    if tt in dtiles or tt >= plan.ne // 128:
                        return
                    dt = dpool.tile([P, 128], bf16, tag="dt", bufs=3,
                                    name="dt")
                    nc.sync.dma_start(out=dt[:],
                                      in_=didT[tt * P:(tt + 1) * P, :])
                    pt = dpool.tile([P, 128], mybir.dt.uint8, tag="pt",
                                    bufs=3, name="pt")
                    nc.sync.dma_start(out=pt[:],
                                      in_=parT[tt * P:(tt + 1) * P, :])
                    dtiles[tt] = (dt, pt)

                def get_ohpar(entry):
                    """one-hot [P, P] bf16 + (par tile, k) for entry."""
                    tt = entry // 128
                    if entry % OHB == 0:
                        load_dtile(tt)
                        load_dtile(tt + 1)     # prefetch next did/par tile
                        dt = dtiles[tt][0]
                        k = entry % 128
                        oh = ohpool.tile([P, OHB * P], bf16, tag="oh", bufs=4,
                                         name="oh")
                        iota_b = io_t[:].rearrange(
                            "p (a c) -> p a c", a=1).to_broadcast([P, OHB, P])
                        did_b = dt[:, k:k + OHB].rearrange(
                            "p (b c) -> p b c", c=1).to_broadcast([P, OHB, P])
                        nc.vector.tensor_tensor(
                            out=oh[:].rearrange("p (b c) -> p b c", c=P),
                            in0=iota_b, in1=did_b,
                            op=mybir.AluOpType.is_equal)
                        dstate["oh"], dstate["oh_e"] = oh, entry
                    k = entry - dstate["oh_e"]
                    return (dstate["oh"][:, k * P:(k + 1) * P],
                            dtiles[tt][1], entry % 128)

                # enumerate all columns in consumption order
                cols = []
                for w in range(nwin):
                    for h in range(2):
                        base = int(plan.stream_off[h, w])
                        for cc in range(int(plan.pwh[h, w]) // P):
                            cols.append((w, h, base + cc * P))
                n_mm_w = [int(plan.pwh[:, w].sum()) // P for w in range(nwin)]

                cur_w = [-1]
                mm_i = [0]
                acc = [None]

                out_q = []

                def flush_batch(batch, tp, tr):
                    # ACT: PSUM -> SBUF bf16 copy of the whole batch
                    nb = len(batch)
                    nc.scalar.activation(out=tr[:, :nb * P],
                                         in_=tp[:, :nb * P], func=Copy)
                    # DVE: in-place parity resolution for the batch
                    _, pt0, ke0 = batch[0][3]
                    nc.vector.copy_predicated(
                        out=tr[:].rearrange("p (b c) -> p b c",
                                            c=P)[:, 0:nb, 0:ch],
                        mask=pt0[:, ke0:ke0 + nb].rearrange(
                            "p (b c) -> p b c", c=1).to_broadcast([P, nb, ch]),
                        data=tr[:].rearrange("p (b c) -> p b c",
                                             c=P)[:, 0:nb, ch:2 * ch])
                    # PE: one one-hot matmul per column
                    for jb, (w, h, pos, ohk) in enumerate(batch):
                        oh = ohk[0]
                        if w != cur_w[0]:
                            cur_w[0] = w
                            mm_i[0] = 0
                            acc[0] = pspool.tile([P, CH1], f32, tag="win",
                                                 bufs=2, name="win")
                        nc.tensor.matmul(
                            out=acc[0][:, :ch], lhsT=oh,
                            rhs=tr[:, jb * P:jb * P + ch],
                            start=(mm_i[0] == 0),
                            stop=(mm_i[0] == n_mm_w[w] - 1))
                        mm_i[0] += 1
                        if mm_i[0] == n_mm_w[w]:
                            out_q.append((w, acc[0]))

                batch, tp, tr = [], None, None
                prev = None            # one-batch flush delay: PE does batch
                for cg, (w, h, pos) in enumerate(cols):   # b's transposes
                    s0, _s1, gt = lookup(h, pos)          # before batch b-1's
                    if len(gst[h]["q"]) < 2:              # matmuls
                        emit_chunk(h)      # prefetch one chunk ahead
                    j = (pos - s0) // P
                    jb = cg % TB
                    if jb == 0:
                        tp = pspool.tile([P, TB * P], bf16, tag="tp", bufs=2,
                                         name="tp")
                        tr = rpool.tile([P, TB * P], bf16, tag="tr", bufs=3,
                                        name="tr")
                        batch = []
                    nc.tensor.transpose(out=tp[:, jb * P:(jb + 1) * P],
                                        in_=gt[:, j * P:(j + 1) * P],
                                        identity=id_t[:])
                    batch.append((w, h, pos, get_ohpar(cg)))
                    if jb == TB - 1:
                        if prev is not None:
                            flush_batch(*prev)
                            for item in out_q:
                                yield item
                            out_q.clear()
                        prev = (batch, tp, tr)
                        batch = []
                if prev is not None:
                    flush_batch(*prev)
                if batch:
                    flush_batch(batch, tp, tr)
                for item in out_q:
                    yield item
                out_q.clear()

            def emit_l1(tabs):
                for w, ps in propagate(1, tabs):
                    t1 = epool.tile([P, CH1], f32, tag="t1", bufs=3, name="t1")
                    nc.scalar.activation(out=t1[:], in_=ps[:], func=Copy,
                                         scale=dW_t[:, w:w + 1])
                    t2 = epool.tile([P, CH1], f32, tag="t2", bufs=3, name="t2")
                    nc.vector.tensor_tensor(out=t2[:], in0=t1[:], in1=b1_t[:],
                                            op=mybir.AluOpType.add)
                    h = epool.tile([P, CH1], bf16, tag="h", bufs=3, name="h")
                    nc.vector.tensor_scalar_max(out=h[:], in0=t2[:],
                                                scalar1=0.0)
                    pt = pspool.tile([P, P], bf16, tag="ep", bufs=2,
                                     name="ep1")
                    nc.tensor.transpose(out=pt[0:CH1, :], in_=h[:],
                                        identity=id_t[:])
                    hT = epool.tile([CH1, P], bf16, tag="hT", bufs=3,
                                    name="hT")
                    nc.scalar.activation(out=hT[:], in_=pt[0:CH1, :],
                                         func=Copy)
                    pg = pspool.tile([P, P], f32, tag="bld", bufs=2,
                                     name="ep2")
                    nc.tensor.matmul(out=pg[:, 0:CH2], lhsT=hT[:],
                                     rhs=Wc_t[:], start=True, stop=True)
                    g2s = epool.tile([P, CH2], bf16, tag="g2s", bufs=3,
                                     name="g2s")
                    nc.scalar.activation(out=g2s[:], in_=pg[:, 0:CH2],
                                         func=Copy, scale=dW_t[:, w:w + 1])
                    nc.sync.dma_start(out=g2c[w * P:(w + 1) * P, :],
                                      in_=g2s[:])

            def emit_gonly(tabs):
                """Back-to-back gathers only (throughput microbench)."""
                for h in range(2):
                    for ck, n in enumerate(plan.chunks[h]):
                        ix = ipool.tile([P, GCHUNK // 16], i16,
                                        tag=f"ix{h}", bufs=2, name=f"ix{h}")
                        nc.sync.dma_start(out=ix[:],
                                          in_=idxs[h][ck * P:(ck + 1) * P, :])
                        gt = gpool.tile([P, GCHUNK], bf16,
                                        tag=f"g{h}", bufs=2, name=f"g{h}")
                        nc.gpsimd.dma_gather(
                            out_ap=gt[:, :n].rearrange("p (a n) -> p a n",
                                                       a=1),
                            in_ap=tabs[h][:],
                            idxs_ap=ix[:, :n // 16],
                            num_idxs=n, num_idxs_reg=n, elem_size=128,
                            transpose=True, single_packet=False, queue_num=0,
                            sbuf_tokens_per_rank=128,
                            sbuf_free_dim_per_rank=256,
                            sbuf_free_dim_pad_per_rank=0,
                            sbuf_byte_offset=0)

            def emit_ag():
                nc.gpsimd.collective_compute(
                    "AllGather", mybir.AluOpType.bypass,
                    replica_groups=[list(range(n_cores))],
                    ins=[g2c[:, :]], outs=[g2f[:, :]])

            def emit_l2(tabs):
                for w, ps in propagate(2, tabs):
                    o1 = epool.tile([P, CH2], f32, tag="o1", bufs=3, name="o1")
                    nc.scalar.activation(out=o1[:], in_=ps[:, :CH2], func=Copy,
                                         scale=dW_t[:, w:w + 1])
                    o2 = epool.tile([P, CH2], f32, tag="o2", bufs=3, name="o2")
                    nc.vector.tensor_tensor(out=o2[:], in0=o1[:], in1=bc_t[:],
                                            op=mybir.AluOpType.add)
                    nc.sync.dma_start(out=out_c[w * P:(w + 1) * P, :],
                                      in_=o2[:])

            def emit_pipeline():
                tabs = None
                if 'b1' in stages:
                    tabs = emit_b1()
                if 'g' in stages:
                    emit_gonly(tabs if tabs is not None else table_tiles())
                if 'l1' in stages:
                    emit_l1(tabs if tabs is not None else table_tiles())
                if 'ag' in stages and rep == 1:
                    emit_ag()
                if 'b2' in stages:
                    tabs = emit_b2()
                if 'l2' in stages:
                    emit_l2(tabs if tabs is not None else table_tiles())

            if rep == 1:
                emit_pipeline()
            else:
                for _ in range(n_ag):
                    if 'ag' in stages:
                        emit_ag()
                if any(t in stages for t in ('b1', 'l1', 'b2', 'l2')):
                    with tc.For_i(0, rep, 1) as _i:
                        emit_pipeline()

    nc.compile()
    return nc


def make_in_maps(plan, x, W1, b1, W_mu, b_mu, W_ls, b_ls):
    n_nodes = np.asarray(x).shape[0]
    npad, shard, R = plan.npad, plan.shard, plan.ranks_half
    pairs_half = plan.pairs_half
    totR = npad // 256

    # permuted xT: column (H,g,t,j) <- node 2*(H*pairs_half + g*128 + j) + t
    HH, gg, tt, jj = np.meshgrid(np.arange(2), np.arange(R), np.arange(2),
                                 np.arange(P), indexing="ij")
    nodes_perm = (2 * (HH * pairs_half + gg * P + jj) + tt).reshape(-1)
    xTfull = np.zeros((P, npad), np.float32)
    xTfull[:, :n_nodes] = np.asarray(x, np.float32).T
    xTp = xTfull[:, nodes_perm]
    XB = 3584 if npad % 3584 == 0 else 512
    nxb = npad // XB
    xTf = xTp.reshape(P, nxb, XB).transpose(1, 0, 2).reshape(nxb * P, XB)
    xTf = _to_bf16(xTf)

    # disP[j, 2*g + t] = dis[2*(g*128 + j) + t]  (g = global rank)
    gg2, tt2 = np.meshgrid(np.arange(totR), np.arange(2), indexing="ij")
    disP = np.zeros((P, 2 * totR), np.float32)
    for j in range(P):
        disP[j, :] = plan.dis[(2 * (gg2 * P + j) + tt2).reshape(-1)]

    Wc = np.concatenate([np.asarray(W_mu, np.float32),
                         np.asarray(W_ls, np.float32)], axis=1)
    bc = np.concatenate([np.asarray(b_mu, np.float32),
                         np.asarray(b_ls, np.float32)])
    iota = np.tile(np.arange(P, dtype=np.float32), (P, 1))
    ident = np.eye(P, dtype=np.float32)

    def chunk_idx(stream, chunks):
        tiles = []
        s0 = 0
        for n in chunks:
            buf = np.zeros((P, GCHUNK // 16), np.int16)
            buf[:, :n // 16] = _wrap16(stream[s0:s0 + n])
            tiles.append(buf)
            s0 += n
        return np.concatenate(tiles, axis=0)

    def chunk_did(a):  # [P, ne] -> [(ne/128)*P, 128]
        ne = a.shape[1]
        return a.reshape(P, ne // 128, 128).transpose(1, 0, 2).reshape(
            (ne // 128) * P, 128)

    in_maps = []
    for c in range(plan.n_cores):
        idx, da, pa = plan.core_arrays(c)
        m = {
            "xT": xTf,
            "disP": disP,
            "W1": _to_bf16(np.asarray(W1, np.float32)),
            "Wc": _to_bf16(Wc),
            "b1r": np.tile(np.asarray(b1, np.float32), (P, 1)),
            "bcr": np.tile(bc, (P, 1)),
            "iot": _to_bf16(iota),
            "idn": _to_bf16(ident),
            "disW": plan.dis[c * shard:(c + 1) * shard].reshape(
                plan.nwin, P).T.copy(),
            "didT": _to_bf16(chunk_did(da)),
            "parT": chunk_did(pa).astype(np.uint8),
        }
        for h in range(2):
            m[f"idx_{h}"] = chunk_idx(idx[h], plan.chunks[h])
        in_maps.append(m)
    return in_maps


def _to_bf16(a):
    import ml_dtypes
    return a.astype(ml_dtypes.bfloat16)


def kernel(x, edge_index, W1, b1, W_mu, b_mu, W_ls, b_ls):
    from concourse import bass_utils

    x = np.asarray(x, np.float32)
    n_nodes = x.shape[0]
    plan = Plan(n_nodes, N_CORES, np.asarray(edge_index[0], np.int64),
                np.asarray(edge_index[1], np.int64))
    nc = build_program(plan, N_CORES)
    in_maps = make_in_maps(plan, x, W1, b1, W_mu, b_mu, W_ls, b_ls)
    res = bass_utils.run_bass_kernel_spmd(nc, in_maps,
                                          core_ids=list(range(N_CORES)))
    out = np.concatenate([res.results[c]["out_c"] for c in range(N_CORES)],
                         axis=0)
    return (out[:n_nodes, :16].copy(), out[:n_nodes, 16:].copy())


def _numpy_ref(x, ei, W1, b1, W_mu, b_mu, W_ls, b_ls):
    n = x.shape[0]
    src = np.concatenate([ei[0], np.arange(n)])
    dst = np.concatenate([ei[1], np.arange(n)])
    deg = np.bincount(dst, minlength=n)
    dis = 1 / np.sqrt(deg)

    def conv(f, W, b):
        g = dis[:, None] * (f @ W)
        acc = np.zeros((n, W.shape[1]))
        np.add.at(acc, dst, g[src])
        return dis[:, None] * acc + b

    h = np.maximum(conv(x, W1, b1), 0)
    return conv(h, W_mu, b_mu), conv(h, W_ls, b_ls)
